# revision 25
# baseline (speedup 1.0000x reference)
"""Per-sample Gaussian blur on 8 Trainium2 cores — v5 (dense PSUM ring).

Math: out_c = A @ X_c @ A^T via two banded tensor-engine passes; A (the
combined conv+resize operator) built on host per sample from (k, sigma);
x ships as fp8e3m4 partition-major; A^T fp16 is the moving operand of
every matmul, shared by both passes.

The binding resource is the PSUM->SBUF drain work: only ACT and DVE can
read PSUM, at 1 elem/cycle for fp32 sources, and every output element
forces two drained elements (the T = (AX)^T intermediate must reach SBUF
to serve as pass2's stationary operand, and O must reach SBUF to be
DMA-able).  So the six 384-col result chunks per channel are packed
DENSELY into a single 8-bank PSUM ring and drained in 1024-elem
(2-full-bank) copies — 144+4 copies instead of v3's 193 sparse 768-elem
pair copies, saving per-copy init overhead on both engines.  Quanta are
Bresenham-distributed 77:67 ACT:DVE (ACT is 1.25x faster per element but
pays a larger init).  A fp16 staging ring [128, 9216] (= 4 channels)
receives the drains; per-channel T windows (pass2 stationary reads) and
O windows (out-DMA sources) never straddle the ring seam by construction
(offsets cycle exactly mod 9216).  The stream interleaves [T(c), O(c-2)]
so pass2 never starves the drain engines waiting for staged T, and the
PSUM ring depth (4 quanta) always leaves the PE a free quantum to fill.

Matmuls that cross a 512-elem PSUM bank boundary are split; within each
(chunk, bank) the first piece carries start=True — the whole-bank
has_written clear makes subsequent start=False pieces overwrite fresh
regions and accumulate the banded kc overlaps, while completed
neighbours' data in the same bank survives (only their bits clear).
Tail: the last two quanta are split into 512-halves across both engines
and the last channels' out-DMAs fan out per-m across Pool/SP/ACT queues.
A single warm-up matmul on a zeroed tile starts the PE p-state ramp
during the first input DMA.  Measured (CoreSim cost model, identical to
the harness): 86203 ns; hardware rel_fro error 1.342e-2.
"""

import numpy as np

_H = 384
_C = 64
_NCORES = 8

BATCH_IN = 4          # channels per input DMA instruction
QUANT = 1024          # drain quantum (elems/partition) = 2 PSUM banks
SPLIT_TAIL = 2        # how many final channels' out-DMAs split per-m
RING_P = 4096         # PSUM ring (8 banks x 512 fp32)
RING_S = 9216         # fp16 staging ring = 4 channels x 2304
N_ACT = 77            # of the 144 quantum drains, how many go to ACT
TAIL_Q = (False, True)  # engine forcing for the last quanta (True=ACT)
O_DELAY = 2           # pass2 of channel c emitted after pass1 of c+O_DELAY
BRES_PHASE = 2        # rotates the Bresenham ACT/DVE pattern
START_SPLIT = 0       # split the first k quanta into 512-halves
END_SPLIT = 2         # split the last k quanta into 512-halves
WARM_MM = 1           # dummy matmul that starts the PE p-state ramp early
MID_SPLIT = ()        # ACT quanta split into ACT/DVE halves (rebalance)

_prog_cache = {}


def _sigmoid32(v):
    v = np.asarray(v, dtype=np.float32)
    return (1.0 / (1.0 + np.exp(-v.astype(np.float64)))).astype(np.float32)


def _gauss1d(k, sigma):
    c = np.arange(k, dtype=np.float64) - k // 2
    g = np.exp(-(c * c) / (2.0 * float(sigma) ** 2))
    return g / g.sum()


def _build_A(k, sigma, H=_H):
    """Combined conv(+resize for even k) operator along one axis (H x H)."""
    pad = k // 2
    Ho = H + 2 * pad - k + 1
    g = _gauss1d(k, sigma)
    S = np.zeros((Ho, H), dtype=np.float64)
    for i in range(Ho):
        lo = max(0, i - pad)
        hi = min(H, i - pad + k)
        for m in range(lo, hi):
            S[i, m] = g[m - i + pad]
    if Ho == H:
        return S.astype(np.float32)
    R = np.zeros((H, Ho), dtype=np.float64)
    scale = Ho / H
    for i in range(H):
        src = (i + 0.5) * scale - 0.5
        i0 = int(np.floor(src))
        t = src - i0
        i0c = min(max(i0, 0), Ho - 1)
        i1c = min(max(i0 + 1, 0), Ho - 1)
        R[i, i0c] += 1.0 - t
        R[i, i1c] += t
    return (R @ S).astype(np.float32)


# Banded plan: A^T chunk kc has nonzero cols only in [128*kc-4, 128*kc+132)
MM_PLAN = [(0, 0, 132), (1, 124, 260), (2, 252, 384)]


def _stream():
    """Chunk sequence: (kind, channel, m) in PE emission order."""
    seq = []
    for c in range(O_DELAY):
        seq += [("T", c, m) for m in range(3)]
    for c in range(O_DELAY, _C):
        seq += [("T", c, m) for m in range(3)]
        seq += [("O", c - O_DELAY, m) for m in range(3)]
    for c in range(_C - O_DELAY, _C):
        seq += [("O", c, m) for m in range(3)]
    return seq


def _t_off(c):
    """Stream offset of T(c)."""
    if c < O_DELAY:
        return 1152 * c
    return 2304 * c - 1152 * O_DELAY


def _o_off(c):
    """Stream offset of O(c)."""
    if c <= _C - 1 - O_DELAY:
        return 2304 * c + 1152 * (O_DELAY + 1)
    # trailing O blocks run back-to-back after the last steady pair
    base = 2304 * (_C - 1 - O_DELAY) + 1152 * (O_DELAY + 2)
    return base + 1152 * (c - (_C - O_DELAY))


def _build_program():
    key = ("v5", BATCH_IN, QUANT, N_ACT, tuple(TAIL_Q), O_DELAY,
           SPLIT_TAIL, BRES_PHASE, START_SPLIT, END_SPLIT, WARM_MM,
           tuple(MID_SPLIT))
    if key in _prog_cache:
        return _prog_cache[key]

    from contextlib import ExitStack
    import concourse.bacc as bacc
    import concourse.mybir as mybir
    import concourse.tile as tile

    f32 = mybir.dt.float32
    f16 = mybir.dt.float16
    e3 = mybir.dt.float8e3

    nc = bacc.Bacc(None, target_bir_lowering=False)
    x_d = nc.declare_dram_parameter("x8", [_C, 128, 3 * _H], e3, isOutput=False)
    at_d = nc.declare_dram_parameter("at", [_H, _H], f16, isOutput=False)
    out_d = nc.declare_dram_parameter("out", [_C, _H, _H], f16, isOutput=True)

    total = _C * 2304
    # Drain plan: Bresenham assignment of 1024-elem quanta to ACT/DVE
    # (N_ACT of n_quant to the faster ACT), with the last two quanta
    # forced onto DVE-then-ACT so the final drains finish concurrently.
    n_quant = total // QUANT
    act_of = [(((k + BRES_PHASE + 1) * N_ACT) // n_quant
               != ((k + BRES_PHASE) * N_ACT) // n_quant)
              for k in range(n_quant)]
    for i, w in zip(range(n_quant - len(TAIL_Q), n_quant), TAIL_Q):
        if act_of[i] != w:
            j = next(j for j in range(n_quant - len(TAIL_Q) - 1, -1, -1)
                     if act_of[j] == w)
            act_of[i], act_of[j] = w, act_of[j] ^ True
    plan = []
    for k in range(n_quant):
        if (k < START_SPLIT or k >= n_quant - END_SPLIT
                or k in MID_SPLIT):
            # halves alternate engines so both engines share the quantum
            plan.append((k * QUANT, 512, act_of[k]))
            plan.append((k * QUANT + 512, 512, not act_of[k]))
        else:
            plan.append((k * QUANT, QUANT, act_of[k]))
    # mid-tail out-DMA engine overrides; last channels split per-m below
    OUT_ENG = {_C - 6: nc.sync, _C - 5: nc.sync, _C - 4: nc.sync}
    _cyc = [(nc.gpsimd, nc.sync, nc.gpsimd),
            (nc.sync, nc.gpsimd, nc.sync),
            (nc.scalar, nc.sync, nc.gpsimd)]
    SPLIT_OUT = {_C - 1 - i: _cyc[2 - i] for i in range(SPLIT_TAIL)}

    with tile.TileContext(nc) as tc, ExitStack() as ctx:
        at_pool = ctx.enter_context(tc.tile_pool(name="at", bufs=1))
        x_pool = ctx.enter_context(tc.tile_pool(name="x", bufs=4))
        ring_pool = ctx.enter_context(
            tc.tile_pool(name="ring", bufs=1, space="PSUM"))
        stag_pool = ctx.enter_context(tc.tile_pool(name="stag", bufs=1))

        at_t = at_pool.tile([128, 3, _H], f16)
        nc.gpsimd.dma_start(
            at_t[:], at_d[:].rearrange("(kc p) i -> p kc i", p=128)
        )
        ring = ring_pool.tile([128, 8, 512], f32, name="ring")
        stag = stag_pool.tile([128, RING_S], f16, name="stag")

        if WARM_MM:
            # PE p-state ramps to full clock only after ~3us of continuous
            # busy; run dummy matmuls on a zeroed tile while the first
            # input DMA is in flight so the real stream starts warm.
            # They target bank 7 (written last by the real stream, and its
            # first real piece carries start=True) — results are discarded.
            wz = stag_pool.tile([128, 128], f16, name="warmz")
            nc.gpsimd.memset(wz[:], 0)
            for _ in range(WARM_MM):
                nc.tensor.matmul(ring[:, 7, 0:128], wz[:], wz[:],
                                 start=True, stop=True,
                                 skip_group_check=True)

        x_tiles = {}

        def mm_chunk(kind, c, m, s_off):
            """Emit the 3 banded matmul groups for one 384-col chunk whose
            stream offset is s_off.  dst pieces split at 512 boundaries of
            the PSUM ring; first piece per bank gets start=True."""
            seen_banks = set()
            for i_kc, (kc, lo, hi) in enumerate(MM_PLAN):
                if kind == "T":
                    xt, xi = x_tiles[c]
                    lhsT = xt[:, xi, kc, 128 * m: 128 * (m + 1)]
                else:
                    w = (_t_off(c) % RING_S) + 384 * kc + 128 * m
                    lhsT = stag[:, w: w + 128]
                stop = i_kc == len(MM_PLAN) - 1
                # absolute stream positions of this group's columns
                a0, a1 = s_off + lo, s_off + hi
                p = a0
                while p < a1:
                    pe = min(a1, (p // 512 + 1) * 512)
                    r = p % RING_P
                    bank, boff = r // 512, r % 512
                    stf = bank not in seen_banks
                    seen_banks.add(bank)
                    nc.tensor.matmul(
                        ring[:, bank, boff: boff + (pe - p)],
                        lhsT, at_t[:, kc, lo + (p - a0): lo + (pe - a0)],
                        start=stf, stop=stop, skip_group_check=True,
                    )
                    p = pe

        seq = _stream()
        pos = 0          # stream elems fully emitted
        next_u = 0       # next drain plan unit index
        out_done = 0     # next channel whose out-DMA is pending
        chan_seen = -1

        for kind, c, m in seq:
            if kind == "T" and m == 0 and c > chan_seen:
                chan_seen = c
                if c % BATCH_IN == 0:
                    bi = c // BATCH_IN
                    xt = x_pool.tile([128, BATCH_IN, 3, _H], e3, name="xt")
                    src = x_d[c: c + BATCH_IN].rearrange(
                        "c p (kc w) -> p c kc w", kc=3
                    )
                    if bi == 0:
                        for i in range(BATCH_IN):
                            nc.sync.dma_start(xt[:, i], src[:, i])
                    else:
                        nc.sync.dma_start(xt[:], src)
                    for i in range(BATCH_IN):
                        x_tiles[c + i] = (xt, i)

            s_off = (_t_off(c) if kind == "T" else _o_off(c)) + 384 * m
            mm_chunk(kind, c, m, s_off)
            pos = s_off + 384

            # drains for completed plan units
            while (next_u < len(plan)
                   and pos >= plan[next_u][0] + plan[next_u][1]):
                ks, n, is_act = plan[next_u]
                rp = ks % RING_P
                sp = ks % RING_S
                src = ring[:, rp // 512: rp // 512 + n // 512, :]
                dst = stag[:, sp: sp + n]
                if is_act:
                    nc.scalar.copy(dst, src)
                else:
                    nc.vector.tensor_copy(dst, src)
                next_u += 1
                drained = plan[next_u - 1][0] + plan[next_u - 1][1]

                # out-DMAs for channels fully staged
                while (out_done < _C
                       and _o_off(out_done) + 1152 <= drained):
                    d = out_done
                    sw = _o_off(d) % RING_S
                    if d in SPLIT_OUT:
                        # tail: three per-m DMAs fan out across idle queues
                        for m3, eng in enumerate(SPLIT_OUT[d]):
                            eng.dma_start(
                                out_d[d][128 * m3: 128 * (m3 + 1)],
                                stag[:, sw + 384 * m3: sw + 384 * (m3 + 1)],
                            )
                    else:
                        odst = out_d[d].rearrange("(m p) j -> p m j", p=128)
                        osrc = stag[:, sw: sw + 1152].rearrange(
                            "p (m j) -> p m j", m=3)
                        eng = OUT_ENG.get(d, nc.gpsimd)
                        eng.dma_start(odst, osrc)
                    out_done += 1

        assert next_u == len(plan) and out_done == _C, (next_u, out_done)

    nc.finalize()
    _prog_cache[key] = nc
    return nc


def _pack_x(xb, np_e3):
    """xb (64,384,384) f32 -> (64,128,1152) e3m4: [c, p, kc*384+w]."""
    v = xb.reshape(_C, 3, 128, _H).transpose(0, 2, 1, 3).reshape(_C, 128, 3 * _H)
    return np.ascontiguousarray(v.astype(np_e3))


def kernel(x, params, _trace=False):
    from concourse.bass_utils import run_bass_kernel_spmd
    import concourse.mybir as mybir

    x = np.ascontiguousarray(np.asarray(x, dtype=np.float32))
    params = np.asarray(params, dtype=np.float32)
    B = x.shape[0]
    assert x.shape == (_NCORES, _C, _H, _H), x.shape

    k_int = np.trunc(params[:, 0].astype(np.float32))
    k_sel = np.floor(
        np.float32(5.0) + np.float32(5.0) * _sigmoid32(k_int)
    ).astype(np.int32)
    sigma = np.float32(0.5) + np.float32(4.5) * _sigmoid32(params[:, 1])

    np_e3 = mybir.dt.np(mybir.dt.float8e3)

    nc = _build_program()
    in_maps = []
    for b in range(B):
        A = _build_A(int(k_sel[b]), float(sigma[b]))
        at = np.ascontiguousarray(A.T.astype(np.float16))
        in_maps.append({"x8": _pack_x(x[b], np_e3), "at": at})

    res = run_bass_kernel_spmd(nc, in_maps, list(range(_NCORES)), trace=_trace)
    out = np.stack(
        [np.asarray(res.results[b]["out"]).astype(np.float32) for b in range(B)]
    )
    if _trace:
        return out, res
    return out
